# revision 13
# baseline (speedup 1.0000x reference)
"""Trainium2 Bass kernel for nn_CalculateAttention (B=2, H=16, S=2048, D=64, fp32).

Strategy: shard the 32 (batch*head) attention instances across 8 cores (4 per
core, processed as 2 pairs); each core computes full attention for its heads.

v2 design (ACT-bound):
  - All device-side operands in fp16 (host converts). fp16 matmuls stream
    1 col/cycle AND allow a separate LDWEIGHTS instruction (fp32/fp32r
    matmuls must self-load their weights inside the MATMUL, which serializes
    the weight load with the stream and defeats row-packing).
  - MM1 (scores, S^T layout so softmax's reduction is a matmul contraction):
    the two heads of a pair are row-packed — head0's K-tile weights occupy PE
    rows 0-63, head1's rows 64-127, so their K=64 matmuls run concurrently.
    Both write one merged PSUM tile st[128, 1024] (cols 0-511 head0,
    512-1023 head1), qchunk=512.
  - ACT: one Exp instruction per k-step over the merged tile (FD=1024,
    scale=1/sqrt(D) fused). ScalarE is the bottleneck engine: its 1 elem/
    cycle/lane @ 1.2 GHz over S^2 scores x 4 heads ~= 109 us/core is the
    floor of this algorithm; everything else is hidden behind it.
  - MM2: per head, lhsT = V''[k-tile, 65] (V with a ones column appended: row
    64 of the accumulator becomes the softmax denominator), rhs = exp tile,
    fp32 PSUM accumulation over the 16 k-tiles. Runs one k-step behind
    MM1/ACT (software pipeline).
  - Epilogue (all on DVE/DMA, off the ACT critical path): copy denominator
    row to SBUF, reciprocal_approx_fast, DRAM-bounce partition-broadcast,
    multiply straight out of PSUM, DMA out as O^T[d, q].
  - PSUM: st [128,1024]f32 = 2 banks x2 bufs + acc0/acc1 [65,512]f32 = 1
    bank x2 tags x2 bufs = 8/8 banks.
Host side only reshapes/transposes/casts (layout prep + unshard).
"""

import numpy as np

_B, _H, _S, _D = 2, 16, 2048, 64
_NCORES = 8
_HPC = (_B * _H) // _NCORES  # heads per core
_QCHUNK = 512  # q columns per head per score tile
_KTILE = 128  # k rows per score tile (partition dim)

_nc_cache = None


def _build_nc(hpc=_HPC, s=_S, d=_D, qchunk=_QCHUNK, reps=1, mode="full"):
    import concourse.bacc as bacc
    import concourse.tile as tile
    from concourse import mybir

    assert hpc % 2 == 0, "heads processed in pairs"
    fp32 = mybir.dt.float32
    fp16 = mybir.dt.float16
    n_k = s // _KTILE
    n_qc = s // qchunk
    scale = 1.0 / float(np.sqrt(np.float32(d)))

    nc = bacc.Bacc("TRN2")
    # Q^T/K^T with head pairs stacked along the partition dim: [pair, 2*d, s]
    QT = nc.dram_tensor("QT", [hpc // 2, 2 * d, s], fp16, kind="ExternalInput")
    KT = nc.dram_tensor("KT", [hpc // 2, 2 * d, s], fp16, kind="ExternalInput")
    # V'' = [V | ones], host-prepared in [head, k%128, k//128, d+1] layout
    V = nc.dram_tensor("V", [hpc, _KTILE, n_k, d + 1], fp16, kind="ExternalInput")
    OT = nc.dram_tensor("OT", [hpc, d, s], fp32, kind="ExternalOutput")

    with tile.TileContext(nc) as tc:
        with (
            tc.tile_pool(name="qk", bufs=2) as qk_pool,
            tc.tile_pool(name="vp", bufs=4) as v_pool,
            tc.tile_pool(name="exp", bufs=3) as exp_pool,
            tc.tile_pool(name="outp", bufs=2) as out_pool,
            tc.tile_pool(name="small", bufs=3) as small_pool,
            tc.tile_pool(name="ps_st", bufs=2, space="PSUM") as ps_st,
            tc.tile_pool(name="ps_acc", bufs=2, space="PSUM") as ps_acc,
            tc.tile_pool(name="dram", bufs=4, space="DRAM") as dram_pool,
        ):

            def epilogue(acc, h, q0, nsplit=1):
                # all off the ACT critical path: DVE + DMA only.
                # DVE's iterative divide is ~8 cyc/elem on a single-partition
                # row; reshape the denominator to [128, q/128] via a DRAM
                # bounce so the reciprocal runs wide. nsplit>1 pipelines the
                # chain in q-slices (used on the final tiles to cut the
                # exposed tail).
                cq = qchunk // nsplit
                for sp in range(nsplit):
                    ss = slice(sp * cq, (sp + 1) * cq)
                    dn = small_pool.tile([1, cq], fp32, tag="dn")
                    nc.vector.tensor_copy(dn, acc[d : d + 1, ss])
                    dnd = dram_pool.tile([1, cq], fp32, tag="dnd")
                    nc.sync.dma_start(out=dnd, in_=dn)
                    denw = small_pool.tile([128, cq // 128], fp32, tag="denw")
                    nc.sync.dma_start(
                        out=denw, in_=dnd.rearrange("o (p j) -> (o p) j", p=128)
                    )
                    recw = small_pool.tile([128, cq // 128], fp32, tag="recw")
                    nc.vector.reciprocal(out=recw, in_=denw)
                    dscr = dram_pool.tile([1, cq], fp32, tag="dscr")
                    nc.sync.dma_start(
                        out=dscr.rearrange("o (p j) -> (o p) j", p=128), in_=recw
                    )
                    # replicate recip row across d partitions via a DRAM bounce
                    # (DRAM-source SWDGE DMA allows partition-stride-0 reads)
                    bc = small_pool.tile([d, cq], fp32, tag="bc")
                    nc.gpsimd.dma_start(out=bc, in_=dscr.to_broadcast((d, cq)))
                    ob = out_pool.tile([d, cq], fp32, tag="ob")
                    nc.vector.tensor_mul(ob, acc[0:d, ss], bc)
                    nc.sync.dma_start(
                        out=OT[h, :, q0 + sp * cq : q0 + (sp + 1) * cq], in_=ob
                    )

            def emit_pair_loads(pair, first):
                """DMA one pair's Q^T/K^T/V into fresh ring buffers. For the
                first pair the chunks are ordered by when MM1 needs them."""
                qt = qk_pool.tile([2 * d, s], fp16, tag="qt")
                kt = qk_pool.tile([2 * d, s], fp16, tag="kt")
                c4 = s // 4
                chunks = (
                    [("k", 0), ("q", 0), ("k", 1), ("k", 2), ("k", 3),
                     ("q", 1), ("q", 2), ("q", 3)]
                    if first
                    else [("k", c) for c in range(4)] + [("q", c) for c in range(4)]
                )
                for which, c in chunks:
                    cs = slice(c * c4, (c + 1) * c4)
                    if which == "k":
                        nc.sync.dma_start(out=kt[:, cs], in_=KT[pair][:, cs])
                    else:
                        nc.sync.dma_start(out=qt[:, cs], in_=QT[pair][:, cs])
                vpp0 = v_pool.tile([_KTILE, n_k, d + 1], fp16, tag="v")
                vpp1 = v_pool.tile([_KTILE, n_k, d + 1], fp16, tag="v")
                nc.sync.dma_start(out=vpp0, in_=V[2 * pair])
                nc.sync.dma_start(out=vpp1, in_=V[2 * pair + 1])
                return qt, kt, vpp0, vpp1

            def emit_body():
                npairs = hpc // 2
                # dummy exp on a tiny tile so walrus's ACT table load (~2.7us)
                # overlaps the initial input DMAs instead of the first score
                warm = small_pool.tile([1, 2], fp32, tag="warm")
                nc.vector.memset(warm, 0.0)
                nc.scalar.activation(
                    out=warm, in_=warm, func=mybir.ActivationFunctionType.Exp
                )
                loads = {0: emit_pair_loads(0, first=True)}
                if mode == "dma":
                    for pair in range(1, npairs):
                        loads[pair] = emit_pair_loads(pair, first=False)
                    return

                def emit_mm1_act(ld, qc, k):
                    qt, kt, _, _ = ld
                    q0 = qc * qchunk
                    k0 = k * _KTILE
                    st = ps_st.tile([_KTILE, 2 * qchunk], fp32, tag="st")
                    # row-packed MM1s: head0 on PE rows 0-63, head1 on
                    # 64-127 -> disjoint row groups, run concurrently
                    nc.tensor.matmul(
                        st[:, 0:qchunk],
                        lhsT=kt[0:d, k0 : k0 + _KTILE],
                        rhs=qt[0:d, q0 : q0 + qchunk],
                        start=True,
                        stop=True,
                    )
                    nc.tensor.matmul(
                        st[:, qchunk : 2 * qchunk],
                        lhsT=kt[d : 2 * d, k0 : k0 + _KTILE],
                        rhs=qt[d : 2 * d, q0 : q0 + qchunk],
                        start=True,
                        stop=True,
                    )
                    if mode == "mm1":
                        return None
                    ex = exp_pool.tile([_KTILE, 2 * qchunk], fp16, tag="ex")
                    nc.scalar.activation(
                        out=ex,
                        in_=st,
                        func=mybir.ActivationFunctionType.Exp,
                        scale=scale,
                    )
                    return ex

                def emit_mm2(ld, accs, k, ex):
                    _, _, vpp0, vpp1 = ld
                    for half, (acc_t, vpp_t) in enumerate(
                        ((accs[0], vpp0), (accs[1], vpp1))
                    ):
                        nc.tensor.matmul(
                            acc_t[:, :],
                            lhsT=vpp_t[:, k, :],
                            rhs=ex[:, half * qchunk : (half + 1) * qchunk],
                            start=(k == 0),
                            stop=(k == n_k - 1),
                        )

                # one flat software-pipelined stream over (pair, qc, k): the
                # MM1/ACT front runs one step ahead of the MM2 back so the PE
                # stream never starves ACT, including across qc/pair bounds
                steps = [
                    (pair, qc, k)
                    for pair in range(npairs)
                    for qc in range(n_qc)
                    for k in range(n_k)
                ]
                prev = None  # (ld, accs, k, ex, h0, h1, q0)
                accs = None
                for pair, qc, k in steps:
                    if k == 0 and qc == n_qc - 1 and pair + 1 < npairs:
                        # prefetch next pair's inputs one qc ahead
                        loads[pair + 1] = emit_pair_loads(pair + 1, first=False)
                    ld = loads[pair]
                    if k == 0 and mode in ("full", "noepi"):
                        accs = (
                            ps_acc.tile(
                                [d + 1, qchunk], fp32, tag="acc0", name="acc0"
                            ),
                            ps_acc.tile(
                                [d + 1, qchunk], fp32, tag="acc1", name="acc1"
                            ),
                        )
                    ex = emit_mm1_act(ld, qc, k)
                    if prev is not None and mode in ("full", "noepi"):
                        p_ld, p_accs, p_k, p_ex, p_h0, p_h1, p_q0 = prev
                        emit_mm2(p_ld, p_accs, p_k, p_ex)
                        if p_k == n_k - 1 and mode == "full":
                            epilogue(p_accs[0], p_h0, p_q0)
                            epilogue(p_accs[1], p_h1, p_q0)
                    prev = (ld, accs, k, ex, 2 * pair, 2 * pair + 1, qc * qchunk)
                if prev is not None and mode in ("full", "noepi"):
                    p_ld, p_accs, p_k, p_ex, p_h0, p_h1, p_q0 = prev
                    emit_mm2(p_ld, p_accs, p_k, p_ex)
                    if mode == "full":
                        epilogue(p_accs[0], p_h0, p_q0, nsplit=2)
                        epilogue(p_accs[1], p_h1, p_q0, nsplit=2)

            if reps == 1:
                emit_body()
            else:
                with tc.For_i(0, reps, 1):
                    emit_body()
    nc.compile()
    return nc


def _shard_inputs(Q, K, V):
    """Full [B,H,S,D] fp32 inputs -> per-core in_maps: pair-stacked transposed
    fp16 Q/K and ones-augmented, DMA-friendly fp16 V layout."""
    bh = _B * _H
    n_k = _S // _KTILE
    Qf = np.ascontiguousarray(
        np.asarray(Q, dtype=np.float32)
        .reshape(bh, _S, _D)
        .transpose(0, 2, 1)
        .reshape(bh // 2, 2 * _D, _S)
    ).astype(np.float16)
    Kf = np.ascontiguousarray(
        np.asarray(K, dtype=np.float32)
        .reshape(bh, _S, _D)
        .transpose(0, 2, 1)
        .reshape(bh // 2, 2 * _D, _S)
    ).astype(np.float16)
    Vf = np.asarray(V, dtype=np.float32).reshape(bh, _S, _D)
    Vf = np.concatenate([Vf, np.ones((bh, _S, 1), np.float32)], axis=2)
    # [bh, S, D+1] -> [bh, k%128, k//128, D+1]
    Vf = np.ascontiguousarray(
        Vf.reshape(bh, n_k, _KTILE, _D + 1).transpose(0, 2, 1, 3)
    ).astype(np.float16)
    hpc2 = _HPC // 2
    in_maps = []
    for c in range(_NCORES):
        in_maps.append(
            {
                "QT": Qf[c * hpc2 : (c + 1) * hpc2],
                "KT": Kf[c * hpc2 : (c + 1) * hpc2],
                "V": Vf[c * _HPC : (c + 1) * _HPC],
            }
        )
    return in_maps


def _unshard_output(results):
    ot = np.concatenate([r["OT"] for r in results], axis=0)  # [32, 64, 2048]
    return np.ascontiguousarray(
        ot.transpose(0, 2, 1).reshape(_B, _H, _S, _D).astype(np.float32)
    )


def kernel(Q, K, V):
    global _nc_cache
    from concourse import bass_utils

    if _nc_cache is None:
        _nc_cache = _build_nc()
    in_maps = _shard_inputs(Q, K, V)
    res = bass_utils.run_bass_kernel_spmd(
        _nc_cache, in_maps, core_ids=list(range(_NCORES))
    )
    return _unshard_output(res.results)
